# revision 1
# baseline (speedup 1.0000x reference)
"""Multi-head causal attention with RoPE on 8 Trainium2 NeuronCores.

Reference computation (B=2, T=2048, C=1024, H=16, Dh=64, fp32):
    qkv = x @ w_qkv + b_qkv ; split q,k,v ; RoPE(q), RoPE(k)
    attn = softmax_causal(q k^T / sqrt(Dh)) @ v ; out = attn @ w_proj + b_proj

Sharding: core c = b*4 + g handles batch b and head group g (heads 4g..4g+3).
Data-parallel over batch, tensor-parallel over heads (w_qkv column-split,
w_proj row-split).  Each core emits a partial [T, C] projection output; the
host sums the 4 per-batch partials and adds b_proj.

Per-core kernel.  All heavy matmuls run in bf16 (1 PE cycle/row, and low
enough power to stay out of the HAM clock throttle that fp32r's multi-pass
mode triggers); accumulation is always fp32 in PSUM, end-to-end rel err
~5e-3 of output absmax.
  - x^T is pre-transposed on the host, so QKV runs with weights stationary
    producing Q^T/K^T directly in [head_dim, T] layout; V in [T, head_dim].
  - biases are folded into the PSUM accumulation as rank-1 (K=1) matmuls.
  - RoPE: rope(q) = q*cos + shift32(q*sin_perm), the partition shift done
    with a constant 128x128 0/1 permutation matmul.
  - attention per head: S^T tile = K_j Q^T (scores transposed, so the
    softmax sum direction matches the PE contraction), exp on ACT with the
    1/sqrt(Dh) scale fused (no max subtraction: scores are ~N(0,1); fp32
    exp cannot overflow), causal via narrowing each k-tile's q-range plus
    one triangular mask multiply on the diagonal 128x128 block.
  - V is augmented with a ones column so the PV matmul also emits the
    softmax denominator; 1/denom computed on ACT as exp(-ln(d)) (DVE
    reciprocal is 8 cycles/elem, ACT splines are 2x1), then broadcast
    across partitions with a stride-0 DMA.
  - projection: per head-pair stationary attn^T tiles vs w_proj rows.
"""

import numpy as np
import ml_dtypes

import concourse.bacc as bacc
import concourse.bass as bass
import concourse.mybir as mybir
from concourse.tile import TileContext
from concourse.bass_utils import run_bass_kernel_spmd

F32 = mybir.dt.float32
BF16 = mybir.dt.bfloat16
NPBF16 = np.dtype(ml_dtypes.bfloat16)

B, T, C = 2, 2048, 1024
H, DH = 16, 64
GH = 4  # heads per core
N_CORES = 8
NCHUNK = C // 128  # 8 contraction chunks
NT = T // 128  # 16 token tiles
NSPAN = T // 512  # 4 query spans
QK_COLS = 2 * GH * DH  # 512 = q cols (256) + k cols (256)
VA = GH * (DH + 1)  # 260 = v cols augmented with ones column per head
EXP = mybir.ActivationFunctionType.Exp
LOG = mybir.ActivationFunctionType.Ln


def _build():
    nc = bacc.Bacc("TRN2", target_bir_lowering=False, debug=False, num_devices=N_CORES)

    xT = nc.dram_tensor("xT", [C, T], BF16, kind="ExternalInput")
    wqk = nc.dram_tensor("wqk", [C, QK_COLS], BF16, kind="ExternalInput")
    wv = nc.dram_tensor("wv", [C, VA], BF16, kind="ExternalInput")
    bqk_d = nc.dram_tensor("bqk", [1, QK_COLS], BF16, kind="ExternalInput")
    bv_d = nc.dram_tensor("bv", [1, VA], BF16, kind="ExternalInput")
    cos_d = nc.dram_tensor("cosT", [128, T], F32, kind="ExternalInput")
    sinp_d = nc.dram_tensor("sinTp", [128, T], F32, kind="ExternalInput")
    perm_d = nc.dram_tensor("perm", [128, 128], BF16, kind="ExternalInput")
    maskT_d = nc.dram_tensor("maskT", [128, 128], BF16, kind="ExternalInput")
    id_d = nc.dram_tensor("id128", [128, 128], BF16, kind="ExternalInput")
    wproj_d = nc.dram_tensor("wproj", [2, 128, C], BF16, kind="ExternalInput")
    out_d = nc.dram_tensor("out", [T, C], F32, kind="ExternalOutput")

    with TileContext(nc) as tc:
        with tc.tile_pool(name="persist", bufs=1) as pers:
            ones = pers.tile([1, 512], BF16, tag="ones")
            nc.vector.memset(ones, 1.0)
            ones_ff = pers.tile([128, 64], F32, tag="ones_ff")
            nc.vector.memset(ones_ff, 1.0)
            ones_r = pers.tile([128, 64], mybir.dt.float32r, tag="ones_r")
            nc.vector.tensor_copy(ones_r, ones_ff)
            cos_sb = pers.tile([128, T], F32, tag="cos")
            nc.gpsimd.dma_start(out=cos_sb, in_=cos_d[:, :])
            sinp_sb = pers.tile([128, T], F32, tag="sinp")
            nc.gpsimd.dma_start(out=sinp_sb, in_=sinp_d[:, :])
            perm_sb = pers.tile([128, 128], BF16, tag="perm")
            nc.gpsimd.dma_start(out=perm_sb, in_=perm_d[:, :])
            mask_sb = pers.tile([128, 128], BF16, tag="maskT")
            nc.gpsimd.dma_start(out=mask_sb, in_=maskT_d[:, :])
            id_sb = pers.tile([128, 128], BF16, tag="id128")
            nc.gpsimd.dma_start(out=id_sb, in_=id_d[:, :])
            bqk_sb = pers.tile([1, QK_COLS], BF16, tag="bqk")
            nc.gpsimd.dma_start(out=bqk_sb, in_=bqk_d[:, :])
            bv_sb = pers.tile([1, VA], BF16, tag="bv")
            nc.gpsimd.dma_start(out=bv_sb, in_=bv_d[:, :])

            # Outputs of phase 1 (live into phase 2/3)
            qkt = []  # 4 tiles [128, T]: Q heads(0,1), Q(2,3), K(0,1), K(2,3)
            for i in range(4):
                t = pers.tile([128, T], BF16, tag="qkt", bufs=4, name=f"qkt{i}")
                qkt.append(t)
            vaug = []  # 16 tiles [128, VA], k-tile-major natural layout V
            for j in range(NT):
                t = pers.tile([128, VA], BF16, tag="vaug", bufs=NT, name=f"vaug{j}")
                vaug.append(t)
            attn = []  # 2 tiles [128, T]: normalized attn^T for head pairs
            for p in range(2):
                t = pers.tile([128, T], BF16, tag="attn", bufs=2, name=f"attn{p}")
                attn.append(t)

            # ---------------- Phase 1: QKV projection + RoPE ----------------
            with (
                tc.tile_pool(name="p1", bufs=1) as p1,
                tc.tile_pool(name="p1ps", bufs=1, space="PSUM") as p1ps,
            ):
                xt = []
                for kc in range(NCHUNK):
                    t = p1.tile([128, T], BF16, tag="xt", bufs=NCHUNK, name=f"xt{kc}")
                    eng = nc.sync if kc % 2 == 0 else nc.scalar
                    eng.dma_start(out=t, in_=xT[128 * kc : 128 * (kc + 1), :])
                    xt.append(t)
                wqk_t = []
                for kc in range(NCHUNK):
                    t = p1.tile(
                        [128, QK_COLS], BF16, tag="wqk", bufs=NCHUNK, name=f"wqk{kc}"
                    )
                    nc.gpsimd.dma_start(out=t, in_=wqk[128 * kc : 128 * (kc + 1), :])
                    wqk_t.append(t)
                wv_t = []
                for kc in range(NCHUNK):
                    t = p1.tile([128, VA], BF16, tag="wv", bufs=NCHUNK, name=f"wv{kc}")
                    nc.gpsimd.dma_start(out=t, in_=wv[128 * kc : 128 * (kc + 1), :])
                    wv_t.append(t)

                # V natural layout: for each token tile, [128 tok, VA cols]
                for it in range(NT):
                    pv = p1ps.tile([128, VA], F32, tag="psv", bufs=2, name="psv")
                    ts = slice(128 * it, 128 * (it + 1))
                    for kc in range(NCHUNK):
                        nc.tensor.matmul(
                            pv, xt[kc][:, ts], wv_t[kc], start=(kc == 0), stop=False
                        )
                    # bias (includes the ones column): pv[t, c] += bv[c]
                    nc.tensor.matmul(
                        pv, ones[0:1, 0:128], bv_sb, start=False, stop=True
                    )
                    nc.vector.tensor_copy(vaug[it], pv)

                # Q^T / K^T col-tiles with fused bias + RoPE
                # (emit K first so attention's S^T matmuls unblock earliest)
                for ct in (2, 3, 0, 1):
                    cs = slice(128 * ct, 128 * (ct + 1))
                    for sp in range(NSPAN):
                        ss = slice(512 * sp, 512 * (sp + 1))
                        pq = p1ps.tile([128, 512], F32, tag="psqk", bufs=2, name="psqk")
                        for kc in range(NCHUNK):
                            nc.tensor.matmul(
                                pq,
                                wqk_t[kc][:, cs],
                                xt[kc][:, ss],
                                start=(kc == 0),
                                stop=False,
                            )
                        nc.tensor.matmul(
                            pq, bqk_sb[0:1, cs], ones, start=False, stop=True
                        )
                        # rope: qkt = pq*cos + perm @ (pq*sin_perm)
                        t2 = p1.tile([128, 512], BF16, tag="t2", bufs=3, name="t2")
                        nc.vector.tensor_mul(t2, pq, sinp_sb[:, ss])
                        pp = p1ps.tile(
                            [128, 512], F32, tag="psperm", bufs=2, name="psperm"
                        )
                        nc.tensor.matmul(pp, perm_sb, t2, start=True, stop=True)
                        nc.vector.tensor_mul(qkt[ct][:, ss], pq, cos_sb[:, ss])
                        nc.vector.tensor_add(qkt[ct][:, ss], qkt[ct][:, ss], pp)

            # ---------------- Phase 2: causal attention -------------------
            with (
                tc.tile_pool(name="p2", bufs=1) as p2,
                tc.tile_pool(name="p2ps", bufs=1, space="PSUM") as p2ps,
            ):
                # Flat chunk stream over (head, span-pass, k-tile): the PV
                # matmul of chunk i is emitted after the S matmul of chunk
                # i+2, so the PE never waits on ACT's exp (whose latency
                # gaps would keep the HAM clock gate at half speed).
                stream = []
                for h in range(GH):
                    for pas in ((0, 1), (2, 3)):
                        for j in range(NT):
                            for s in pas:
                                if s < j // 4:
                                    continue
                                q0 = max(512 * s, 128 * j)
                                stream.append((h, j, s, q0, 512 * (s + 1) - q0))
                pvps = {}  # (h, s) -> psum tile, allocated at first S chunk

                def normalize(h, s):
                    # attn = pv[0:64] * (1/colsum); 1/colsum = exp(-ln d) on
                    # ACT (DVE reciprocal is 8 cyc/elem), broadcast across
                    # partitions with a rank-1 fp32r matmul.
                    ct = h // 2
                    po = (h % 2) * 64
                    pv = pvps.pop((h, s))
                    rln = p2.tile([65, 512], F32, tag="rln", bufs=2, name="rln")
                    nc.scalar.activation(
                        out=rln[64:65, :], in_=pv[64:65, :], func=LOG
                    )
                    r = p2.tile(
                        [65, 512], mybir.dt.float32r, tag="r", bufs=2, name="r"
                    )
                    nc.scalar.activation(
                        out=r[64:65, :], in_=rln[64:65, :], func=EXP, scale=-1.0
                    )
                    rb = p2ps.tile([64, 512], F32, tag="psrb", bufs=1, name="psrb")
                    nc.tensor.matmul(
                        rb, ones_r[64:65, :], r[64:65, :], start=True, stop=True
                    )
                    rbs = p2.tile([64, 512], F32, tag="rbs", bufs=2, name="rbs")
                    nc.vector.tensor_copy(rbs, rb)
                    nc.vector.tensor_mul(
                        attn[ct][po : po + 64, 512 * s : 512 * (s + 1)],
                        pv[0:64, :],
                        rbs,
                    )

                def emit_pv(item):
                    h, j, s, q0, w, et = item
                    nc.tensor.matmul(
                        pvps[(h, s)][:, q0 - 512 * s :],
                        vaug[j][:, 65 * h : 65 * (h + 1)],
                        et[:, :w],
                        start=(j == 0),
                        stop=(j == 4 * s + 3),
                    )
                    if j == 4 * s + 3:
                        normalize(h, s)

                from collections import deque

                pending = deque()
                for h, j, s, q0, w in stream:
                    ct = h // 2
                    po = (h % 2) * 64
                    qt, kt = qkt[ct], qkt[2 + ct]
                    if (h, s) not in pvps:
                        pvps[(h, s)] = p2ps.tile(
                            [65, 512], F32, tag="pspv", bufs=4, name=f"pspv{h}_{s}"
                        )
                    diag = s == j // 4
                    sps = p2ps.tile([128, 512], F32, tag="pss", bufs=3, name="pss")
                    nc.tensor.matmul(
                        sps[:, :w],
                        kt[po : po + 64, 128 * j : 128 * (j + 1)],
                        qt[po : po + 64, q0 : q0 + w],
                        start=True,
                        stop=not diag,
                    )
                    if diag:
                        # fold the causal mask into the accumulation:
                        # S[k, q0:q0+128] += maskT.T @ I = -400 where k > q
                        nc.tensor.matmul(
                            sps[:, :128], mask_sb, id_sb, start=False, stop=True
                        )
                    et = p2.tile([128, 512], BF16, tag="et", bufs=6, name="et")
                    nc.scalar.activation(
                        out=et[:, :w], in_=sps[:, :w], func=EXP, scale=0.125
                    )
                    pending.append((h, j, s, q0, w, et))
                    if len(pending) > 2:
                        emit_pv(pending.popleft())
                while pending:
                    emit_pv(pending.popleft())

            # ---------------- Phase 3: output projection ------------------
            with (
                tc.tile_pool(name="p3", bufs=1) as p3,
                tc.tile_pool(name="p3ps", bufs=1, space="PSUM") as p3ps,
            ):
                wproj_sb = []
                for p in range(2):
                    t = p3.tile([128, C], BF16, tag="wproj", bufs=2, name=f"wproj{p}")
                    nc.sync.dma_start(out=t, in_=wproj_d[p, :, :])
                    wproj_sb.append(t)
                for it in range(NT):
                    ts = slice(128 * it, 128 * (it + 1))
                    pp = p3ps.tile([128, C], F32, tag="psproj", bufs=2, name="psproj")
                    for p in range(2):
                        for nh in range(2):
                            ns = slice(512 * nh, 512 * (nh + 1))
                            nc.tensor.matmul(
                                pp[:, ns],
                                attn[p][:, ts],
                                wproj_sb[p][:, ns],
                                start=(p == 0),
                                stop=(p == 1),
                            )
                    ob = p3.tile([128, C], F32, tag="ob", bufs=4, name="ob")
                    if it % 2 == 0:
                        nc.scalar.copy(ob, pp)
                    else:
                        nc.vector.tensor_copy(ob, pp)
                    nc.sync.dma_start(out=out_d[ts, :], in_=ob)

    nc.compile()
    return nc


_NC = None


def _get_nc():
    global _NC
    if _NC is None:
        _NC = _build()
    return _NC


def _rope_tables():
    theta = (10000.0 ** (-np.arange(0, DH, 2, dtype=np.float32) / DH)).astype(
        np.float32
    )
    t = np.arange(T, dtype=np.float32)
    sinusoid = np.outer(t, theta).astype(np.float32)  # [T, DH/2]
    sin = np.concatenate([np.sin(sinusoid), np.sin(sinusoid)], axis=1)  # [T, DH]
    cos = np.concatenate([np.cos(sinusoid), np.cos(sinusoid)], axis=1)
    cosT = cos.T  # [DH, T]
    sinT = sin.T
    # sin_perm[e] = sin[(e+32) % 64]
    idx = (np.arange(DH) + 32) % DH
    sinTp = sinT[idx]
    cos2 = np.ascontiguousarray(np.concatenate([cosT, cosT], axis=0))  # [128, T]
    sinp2 = np.ascontiguousarray(np.concatenate([sinTp, sinTp], axis=0))
    return cos2, sinp2


def _perm_matrix():
    p = np.zeros((128, 128), dtype=np.float32)
    for m in range(128):
        blk = m // 64
        k = blk * 64 + (m % 64 + 32) % 64
        p[k, m] = 1.0
    return p


def _mask_matrices():
    # maskT.T @ I adds -400 to S^T[k, q] where k > q (then exp(0.125*s)=0):
    # maskT[a, b] = -400 where b > a
    maskT = -400.0 * np.triu(np.ones((128, 128), dtype=np.float32), 1)
    return maskT, np.eye(128, dtype=np.float32)


def _bf(a):
    return np.ascontiguousarray(np.asarray(a, dtype=np.float32).astype(NPBF16))


def _prepare_in_maps(x, w_qkv, b_qkv, w_proj):
    x = np.asarray(x, dtype=np.float32)
    w_qkv = np.asarray(w_qkv, dtype=np.float32)
    b_qkv = np.asarray(b_qkv, dtype=np.float32)
    w_proj = np.asarray(w_proj, dtype=np.float32)

    cos2, sinp2 = _rope_tables()
    perm = _bf(_perm_matrix())
    maskT, id128 = _mask_matrices()
    maskT, id128 = _bf(maskT), _bf(id128)
    xTs = [_bf(x[b].T) for b in range(B)]

    in_maps = []
    for c in range(N_CORES):
        b, g = divmod(c, 4)
        h0 = g * GH  # first head of the group
        qcols = w_qkv[:, h0 * DH : (h0 + GH) * DH]
        kcols = w_qkv[:, C + h0 * DH : C + (h0 + GH) * DH]
        wqk = _bf(np.concatenate([qcols, kcols], axis=1))
        wv = np.zeros((C, VA), dtype=np.float32)
        bv = np.zeros((1, VA), dtype=np.float32)
        for j in range(GH):
            src = 2 * C + (h0 + j) * DH
            wv[:, j * 65 : j * 65 + DH] = w_qkv[:, src : src + DH]
            bv[0, j * 65 : j * 65 + DH] = b_qkv[src : src + DH]
            bv[0, j * 65 + DH] = 1.0
        bqk = np.concatenate(
            [b_qkv[h0 * DH : (h0 + GH) * DH], b_qkv[C + h0 * DH : C + (h0 + GH) * DH]]
        ).reshape(1, QK_COLS)
        wproj = np.stack(
            [w_proj[(h0 + 2 * p) * DH : (h0 + 2 * p + 2) * DH, :] for p in range(2)]
        )
        in_maps.append(
            {
                "xT": xTs[b],
                "wqk": wqk,
                "wv": _bf(wv),
                "bqk": _bf(bqk),
                "bv": _bf(bv),
                "cosT": cos2,
                "sinTp": sinp2,
                "perm": perm,
                "maskT": maskT,
                "id128": id128,
                "wproj": _bf(wproj),
            }
        )
    return in_maps


def run(x, w_qkv, b_qkv, w_proj, b_proj, trace=False, tmpdir=None):
    nc = _get_nc()
    in_maps = _prepare_in_maps(x, w_qkv, b_qkv, w_proj)
    res = run_bass_kernel_spmd(
        nc, in_maps, list(range(N_CORES)), trace=trace, tmpdir=tmpdir
    )
    b_proj = np.asarray(b_proj, dtype=np.float32)
    out = np.empty((B, T, C), dtype=np.float32)
    for b in range(B):
        acc = res.results[4 * b]["out"].astype(np.float32)
        for g in range(1, 4):
            acc = acc + res.results[4 * b + g]["out"]
        out[b] = acc + b_proj
    return out, res


def kernel(x, w_qkv, b_qkv, w_proj, b_proj):
    out, _ = run(x, w_qkv, b_qkv, w_proj, b_proj, trace=False)
    return out



# revision 3
# speedup vs baseline: 1.3366x; 1.3366x over previous
"""Multi-head causal attention with RoPE on 8 Trainium2 NeuronCores.

Reference computation (B=2, T=2048, C=1024, H=16, Dh=64, fp32):
    qkv = x @ w_qkv + b_qkv ; split q,k,v ; RoPE(q), RoPE(k)
    attn = softmax_causal(q k^T / sqrt(Dh)) @ v ; out = attn @ w_proj + b_proj

Sharding: core c = b*4 + g handles batch b and head group g (heads 4g..4g+3).
Data-parallel over batch, tensor-parallel over heads (w_qkv column-split,
w_proj row-split).  Each core emits a partial [T, C] projection output; the
host sums the 4 per-batch partials and adds b_proj.

Per-core kernel, organized to keep the PE tensor engine saturated end to end
(the HAM clock gate re-throttles the PE to 1.2 GHz after any idle window, so
PE gaps are doubly expensive):
  - DMA: x^T chunks stream on the sync+scalar queues while weights lead the
    gpsimd queue; rope tables (bf16) and w_proj ride the vector queue.  The
    first QKV matmuls start as soon as chunk 0 lands.
  - Phase 1a computes Q^T/K^T for heads 0-1 CHUNK-major across 8 PSUM banks
    so the PE chases the x DMA instead of stalling on the full tensor.
    RoPE fuses bias into the accumulation (rank-1 matmul), then
    qkt = pq*cos + perm @ (pq*sin_perm), the permutation matmul reusing the
    same PSUM bank in place.
  - Phase 1b computes V in [token, head*65] layout (65th col = ones so the
    PV matmul also emits the softmax denominator).
  - Phase 2 streams attention per head, span by span, in units of TWO
    k-tiles sharing a [128, 1024] two-bank PSUM group: one exp ACTIVATE
    covers both tiles (the ACT engine has a 352-cycle fixed cost per
    instruction, so batching matters), with the 1/sqrt(Dh) scale fused and
    causality via per-tile q-range narrowing plus a triangular mask matmul
    on diagonal tiles.  S^T tiles (scores transposed) make the softmax sum
    direction match the PE contraction.  A 2-unit lookahead queue keeps the
    PE 2 units ahead of the ACT exp.
  - Softmax normalization has NO ACT work (the baseline's exp(-ln(d)) tables
    thrashed ACT_TABLE_LOAD): the denominator row is broadcast across
    partitions with a rank-1 fp32r matmul, inverted with the DVE
    reciprocal_approx_fast op (~18 bits, plenty for bf16 storage), and
    multiplied in on the DVE.
  - Q/K for heads 2-3 are computed BETWEEN the head 0-1 and head 2-3
    attention streams (kc-inner, one flex PSUM bank), and the output
    projection for spans 0-2 is interleaved INTO head 3's stream so the
    [T, C] fp32 result DMA overlaps compute instead of trailing it.
All heavy matmuls run in bf16 (fp32 accumulation in PSUM); end-to-end rel
err ~5e-3 of output absmax.
"""

import numpy as np
import ml_dtypes

import concourse.bacc as bacc
import concourse.bass as bass
import concourse.mybir as mybir
from concourse.tile import TileContext
from concourse.bass_utils import run_bass_kernel_spmd

F32 = mybir.dt.float32
F32R = mybir.dt.float32r
BF16 = mybir.dt.bfloat16
NPBF16 = np.dtype(ml_dtypes.bfloat16)

B, T, C = 2, 2048, 1024
H, DH = 16, 64
GH = 4  # heads per core
N_CORES = 8
NCHUNK = C // 128  # 8 contraction chunks
NT = T // 128  # 16 token tiles
NSPAN = T // 512  # 4 query spans
QK_COLS = 2 * GH * DH  # 512 = q cols (256) + k cols (256)
VA = GH * (DH + 1)  # 260 = v cols augmented with ones column per head
EXP = mybir.ActivationFunctionType.Exp


def _build():
    nc = bacc.Bacc("TRN2", target_bir_lowering=False, debug=False, num_devices=N_CORES)

    xT = nc.dram_tensor("xT", [C, T], BF16, kind="ExternalInput")
    wqk = nc.dram_tensor("wqk", [C, QK_COLS], BF16, kind="ExternalInput")
    wv = nc.dram_tensor("wv", [C, VA], BF16, kind="ExternalInput")
    bqk_d = nc.dram_tensor("bqk", [1, QK_COLS], BF16, kind="ExternalInput")
    bv_d = nc.dram_tensor("bv", [1, VA], BF16, kind="ExternalInput")
    cos_d = nc.dram_tensor("cosT", [128, T], BF16, kind="ExternalInput")
    sinp_d = nc.dram_tensor("sinTp", [128, T], BF16, kind="ExternalInput")
    perm_d = nc.dram_tensor("perm", [128, 128], BF16, kind="ExternalInput")
    maskT_d = nc.dram_tensor("maskT", [128, 128], BF16, kind="ExternalInput")
    id_d = nc.dram_tensor("id128", [128, 128], BF16, kind="ExternalInput")
    wproj_d = nc.dram_tensor("wproj", [2, 128, C], BF16, kind="ExternalInput")
    out_d = nc.dram_tensor("out", [T, C], F32, kind="ExternalOutput")

    with TileContext(nc) as tc:
        with tc.tile_pool(name="persist", bufs=1) as pers:
            # x^T chunks lead the sync/scalar DMA queues: phase 1a chases them.
            xt = []
            for kc in range(NCHUNK):
                t = pers.tile([128, T], BF16, tag="xt", bufs=NCHUNK, name=f"xt{kc}")
                eng = nc.sync if kc % 2 == 0 else nc.scalar
                eng.dma_start(out=t, in_=xT[128 * kc : 128 * (kc + 1), :])
                xt.append(t)
            # Weights lead the gpsimd queue (wqk first: phase 1a is QK),
            # rope tables next (needed at ~15us), wproj last (phase 3).
            wqk_t = []
            for kc in range(NCHUNK):
                t = pers.tile(
                    [128, QK_COLS], BF16, tag="wqk", bufs=NCHUNK, name=f"wqk{kc}"
                )
                nc.gpsimd.dma_start(out=t, in_=wqk[128 * kc : 128 * (kc + 1), :])
                wqk_t.append(t)
            cos_sb = pers.tile([128, T], BF16, tag="cos")
            nc.gpsimd.dma_start(out=cos_sb, in_=cos_d[:, :])
            sinp_sb = pers.tile([128, T], BF16, tag="sinp")
            nc.gpsimd.dma_start(out=sinp_sb, in_=sinp_d[:, :])
            wv_t = []
            for kc in range(NCHUNK):
                t = pers.tile([128, VA], BF16, tag="wv", bufs=NCHUNK, name=f"wv{kc}")
                nc.gpsimd.dma_start(out=t, in_=wv[128 * kc : 128 * (kc + 1), :])
                wv_t.append(t)
            bqk_sb = pers.tile([1, QK_COLS], BF16, tag="bqk")
            nc.gpsimd.dma_start(out=bqk_sb, in_=bqk_d[:, :])
            bv_sb = pers.tile([1, VA], BF16, tag="bv")
            nc.gpsimd.dma_start(out=bv_sb, in_=bv_d[:, :])
            perm_sb = pers.tile([128, 128], BF16, tag="perm")
            nc.gpsimd.dma_start(out=perm_sb, in_=perm_d[:, :])
            mask_sb = pers.tile([128, 128], BF16, tag="maskT")
            nc.gpsimd.dma_start(out=mask_sb, in_=maskT_d[:, :])
            id_sb = pers.tile([128, 128], BF16, tag="id128")
            nc.gpsimd.dma_start(out=id_sb, in_=id_d[:, :])
            wproj_sb = []
            for p in range(2):
                t = pers.tile([128, C], BF16, tag="wproj", bufs=2, name=f"wproj{p}")
                nc.gpsimd.dma_start(out=t, in_=wproj_d[p, :, :])
                wproj_sb.append(t)

            ones = pers.tile([1, 512], BF16, tag="ones")
            nc.vector.memset(ones, 1.0)
            ones_ff = pers.tile([128, 64], F32, tag="ones_ff")
            nc.vector.memset(ones_ff, 1.0)
            ones_r = pers.tile([128, 64], F32R, tag="ones_r")
            nc.vector.tensor_copy(ones_r, ones_ff)

            # Outputs of phase 1 (live into phase 2/3)
            qkt = []  # 4 tiles [128, T]: Q heads(0,1), Q(2,3), K(0,1), K(2,3)
            for i in range(4):
                t = pers.tile([128, T], BF16, tag="qkt", bufs=4, name=f"qkt{i}")
                qkt.append(t)
            vaug = []  # 16 tiles [128, VA], k-tile-major natural layout V
            for j in range(NT):
                t = pers.tile([128, VA], BF16, tag="vaug", bufs=NT, name=f"vaug{j}")
                vaug.append(t)
            attn = []  # 2 tiles [128, T]: normalized attn^T for head pairs
            for p in range(2):
                t = pers.tile([128, T], BF16, tag="attn", bufs=2, name=f"attn{p}")
                attn.append(t)

            # ------------- Phase 1a: Q/K heads 0-1, chunk-major -------------
            # One PSUM bank per (ct, span); all 8 banks live so each arriving
            # x chunk feeds 8 back-to-back matmuls.  ct=2 (K) first so the
            # attention stream's S matmuls unblock earliest.
            combosA = [(ct, sp) for ct in (2, 0) for sp in range(NSPAN)]
            with (
                tc.tile_pool(name="p1a", bufs=1) as p1a,
                tc.tile_pool(name="p1aps", bufs=1, space="PSUM") as p1aps,
            ):
                psqk = [
                    p1aps.tile([128, 512], F32, tag="psqk", bufs=8, name=f"psqk{i}")
                    for i in range(8)
                ]
                for kc in range(NCHUNK):
                    for i, (ct, sp) in enumerate(combosA):
                        cs = slice(128 * ct, 128 * (ct + 1))
                        ss = slice(512 * sp, 512 * (sp + 1))
                        nc.tensor.matmul(
                            psqk[i],
                            wqk_t[kc][:, cs],
                            xt[kc][:, ss],
                            start=(kc == 0),
                            stop=False,
                        )
                # Bias + rope per combo; the perm matmul reuses the combo's
                # own PSUM bank in place once t2/qc have consumed it.
                for i, (ct, sp) in enumerate(combosA):
                    cs = slice(128 * ct, 128 * (ct + 1))
                    ss = slice(512 * sp, 512 * (sp + 1))
                    nc.tensor.matmul(
                        psqk[i], bqk_sb[0:1, cs], ones, start=False, stop=True
                    )
                    t2 = p1a.tile([128, 512], BF16, tag="t2", bufs=3, name="t2")
                    nc.vector.tensor_mul(t2, psqk[i], sinp_sb[:, ss])
                    qc = p1a.tile([128, 512], BF16, tag="qc", bufs=3, name="qc")
                    nc.vector.tensor_mul(qc, psqk[i], cos_sb[:, ss])
                    nc.tensor.matmul(psqk[i], perm_sb, t2, start=True, stop=True)
                    nc.vector.tensor_add(qkt[ct][:, ss], qc, psqk[i])

            # ------------- Phase 1b: V natural layout -----------------------
            with tc.tile_pool(name="p1bps", bufs=1, space="PSUM") as p1bps:
                for it in range(NT):
                    pv = p1bps.tile([128, VA], F32, tag="psv", bufs=2, name="psv")
                    ts = slice(128 * it, 128 * (it + 1))
                    for kc in range(NCHUNK):
                        nc.tensor.matmul(
                            pv, xt[kc][:, ts], wv_t[kc], start=(kc == 0), stop=False
                        )
                    nc.tensor.matmul(
                        pv, ones[0:1, 0:128], bv_sb, start=False, stop=True
                    )
                    nc.vector.tensor_copy(vaug[it], pv)

            # ------------- Phase 2: attention + interleaved extras ----------
            with (
                tc.tile_pool(name="p2", bufs=1) as p2,
                tc.tile_pool(name="p2ps", bufs=1, space="PSUM") as p2ps,
            ):
                pvs = {}
                pending = []
                norm_done = set()

                def normalize(h, s):
                    # attn = pv[0:64] * (1/denom): broadcast the denominator
                    # row across partitions with a rank-1 fp32r matmul, invert
                    # on the DVE (no ACT involvement at all).
                    ct = h // 2
                    po = (h % 2) * 64
                    pv = pvs.pop((h, s))
                    dsb = p2.tile([65, 512], F32R, tag="dsb", bufs=2, name="dsb")
                    nc.vector.tensor_copy(dsb[64:65, :], pv[64:65, :])
                    rb = p2ps.tile([128, 512], F32, tag="flex", bufs=2, name="rb")
                    nc.tensor.matmul(
                        rb[0:64, :], ones_r[64:65, :], dsb[64:65, :],
                        start=True, stop=True,
                    )
                    rbf = p2.tile([64, 512], F32, tag="rbf", bufs=2, name="rbf")
                    nc.vector.reciprocal_approx_fast(rbf, rb[0:64, :])
                    nc.vector.tensor_mul(
                        attn[ct][po : po + 64, 512 * s : 512 * (s + 1)],
                        pv[0:64, :],
                        rbf,
                    )
                    norm_done.add((h, s))

                def emit_pv(item):
                    h, s, parts, et = item
                    for (j, cstart, q0r, w) in parts:
                        nc.tensor.matmul(
                            pvs[(h, s)][:, q0r:512],
                            vaug[j][:, 65 * h : 65 * (h + 1)],
                            et[:, cstart : cstart + w],
                            start=(j == 0),
                            stop=(j == 4 * s + 3),
                        )
                        if j == 4 * s + 3:
                            normalize(h, s)

                def flush():
                    while pending:
                        emit_pv(pending.pop(0))

                def stream_head(h, span_hook=None):
                    ct = h // 2
                    po = (h % 2) * 64
                    qt, kt = qkt[ct], qkt[2 + ct]
                    for s in range(NSPAN):
                        if span_hook is not None:
                            span_hook(h, s)
                        if (h, s) not in pvs:
                            pvs[(h, s)] = p2ps.tile(
                                [65, 512], F32, tag="pv", bufs=2, name=f"pv{h}_{s}"
                            )
                        js = list(range(4 * s + 4))
                        for u in range(0, len(js), 2):
                            j0, j1 = js[u], js[u + 1]
                            sg = p2ps.tile(
                                [128, 1024], F32, tag="sg", bufs=2, name="sg"
                            )
                            parts = []
                            cstart = 0
                            for j in (j0, j1):
                                q0r = max(0, 128 * j - 512 * s)
                                w = 512 - q0r
                                if j != j0 and cstart + w > 512:
                                    cstart = 512
                                diag = j // 4 == s
                                nc.tensor.matmul(
                                    sg[:, cstart : cstart + w],
                                    kt[po : po + 64, 128 * j : 128 * (j + 1)],
                                    qt[po : po + 64, 512 * s + q0r : 512 * (s + 1)],
                                    start=True,
                                    stop=not diag,
                                )
                                if diag:
                                    nc.tensor.matmul(
                                        sg[:, cstart : cstart + 128],
                                        mask_sb,
                                        id_sb,
                                        start=False,
                                        stop=True,
                                    )
                                parts.append((j, cstart, q0r, w))
                                cstart += w
                            et = p2.tile([128, 1024], BF16, tag="et", bufs=4, name="et")
                            nc.scalar.activation(
                                out=et[:, :cstart], in_=sg[:, :cstart],
                                func=EXP, scale=0.125,
                            )
                            pending.append((h, s, parts, et))
                            if len(pending) > 2:
                                emit_pv(pending.pop(0))

                stream_head(0)
                stream_head(1)
                flush()

                # Q/K heads 2-3 between the two attention stream pairs: pure
                # PE work that keeps the HAM clock gate warm while ACT drains.
                for ct, sp in [(3, 0), (1, 0), (3, 1), (1, 1),
                               (3, 2), (1, 2), (3, 3), (1, 3)]:
                    cs = slice(128 * ct, 128 * (ct + 1))
                    ss = slice(512 * sp, 512 * (sp + 1))
                    pq = p2ps.tile([128, 512], F32, tag="flex", bufs=2, name="pqB")
                    for kc in range(NCHUNK):
                        nc.tensor.matmul(
                            pq, wqk_t[kc][:, cs], xt[kc][:, ss],
                            start=(kc == 0), stop=False,
                        )
                    nc.tensor.matmul(
                        pq, bqk_sb[0:1, cs], ones, start=False, stop=True
                    )
                    t2 = p2.tile([128, 512], BF16, tag="t2b", bufs=2, name="t2b")
                    nc.vector.tensor_mul(t2, pq, sinp_sb[:, ss])
                    qc = p2.tile([128, 512], BF16, tag="qcb", bufs=2, name="qcb")
                    nc.vector.tensor_mul(qc, pq, cos_sb[:, ss])
                    nc.tensor.matmul(pq, perm_sb, t2, start=True, stop=True)
                    nc.vector.tensor_add(qkt[ct][:, ss], qc, pq)

                # Output projection for token span s (needs all 4 heads
                # normalized for span s).
                ob_box = {}

                def emit_p3(s):
                    for it in range(4 * s, 4 * s + 4):
                        ts = slice(128 * it, 128 * (it + 1))
                        ob = p2.tile([128, C], F32, tag="ob", bufs=2, name="ob")
                        for nh in range(2):
                            ns = slice(512 * nh, 512 * (nh + 1))
                            pp = p2ps.tile(
                                [128, 512], F32, tag="flex", bufs=2, name="pp3"
                            )
                            for p in range(2):
                                nc.tensor.matmul(
                                    pp,
                                    attn[p][:, ts],
                                    wproj_sb[p][:, ns],
                                    start=(p == 0),
                                    stop=(p == 1),
                                )
                            nc.vector.tensor_copy(ob[:, ns], pp)
                        nc.sync.dma_start(out=out_d[ts, :], in_=ob)

                stream_head(2)

                def h3_hook(h, s):
                    # Two spans back is guaranteed normalized for all heads.
                    if s >= 2:
                        emit_p3(s - 2)

                stream_head(3, span_hook=h3_hook)
                flush()
                emit_p3(2)
                emit_p3(3)

    nc.compile()
    return nc


_NC = None


def _get_nc():
    global _NC
    if _NC is None:
        _NC = _build()
    return _NC


def _rope_tables():
    theta = (10000.0 ** (-np.arange(0, DH, 2, dtype=np.float32) / DH)).astype(
        np.float32
    )
    t = np.arange(T, dtype=np.float32)
    sinusoid = np.outer(t, theta).astype(np.float32)  # [T, DH/2]
    sin = np.concatenate([np.sin(sinusoid), np.sin(sinusoid)], axis=1)  # [T, DH]
    cos = np.concatenate([np.cos(sinusoid), np.cos(sinusoid)], axis=1)
    cosT = cos.T  # [DH, T]
    sinT = sin.T
    # sin_perm[e] = sin[(e+32) % 64]
    idx = (np.arange(DH) + 32) % DH
    sinTp = sinT[idx]
    cos2 = np.ascontiguousarray(np.concatenate([cosT, cosT], axis=0))  # [128, T]
    sinp2 = np.ascontiguousarray(np.concatenate([sinTp, sinTp], axis=0))
    return _bf(cos2), _bf(sinp2)


def _perm_matrix():
    p = np.zeros((128, 128), dtype=np.float32)
    for m in range(128):
        blk = m // 64
        k = blk * 64 + (m % 64 + 32) % 64
        p[k, m] = 1.0
    return p


def _mask_matrices():
    # maskT.T @ I adds -400 to S^T[k, q] where k > q (then exp(0.125*s)=0):
    # maskT[a, b] = -400 where b > a
    maskT = -400.0 * np.triu(np.ones((128, 128), dtype=np.float32), 1)
    return maskT, np.eye(128, dtype=np.float32)


def _bf(a):
    return np.ascontiguousarray(np.asarray(a, dtype=np.float32).astype(NPBF16))


def _prepare_in_maps(x, w_qkv, b_qkv, w_proj):
    x = np.asarray(x, dtype=np.float32)
    w_qkv = np.asarray(w_qkv, dtype=np.float32)
    b_qkv = np.asarray(b_qkv, dtype=np.float32)
    w_proj = np.asarray(w_proj, dtype=np.float32)

    cos2, sinp2 = _rope_tables()
    perm = _bf(_perm_matrix())
    maskT, id128 = _mask_matrices()
    maskT, id128 = _bf(maskT), _bf(id128)
    xTs = [_bf(x[b].T) for b in range(B)]

    in_maps = []
    for c in range(N_CORES):
        b, g = divmod(c, 4)
        h0 = g * GH  # first head of the group
        qcols = w_qkv[:, h0 * DH : (h0 + GH) * DH]
        kcols = w_qkv[:, C + h0 * DH : C + (h0 + GH) * DH]
        wqk = _bf(np.concatenate([qcols, kcols], axis=1))
        wv = np.zeros((C, VA), dtype=np.float32)
        bv = np.zeros((1, VA), dtype=np.float32)
        for j in range(GH):
            src = 2 * C + (h0 + j) * DH
            wv[:, j * 65 : j * 65 + DH] = w_qkv[:, src : src + DH]
            bv[0, j * 65 : j * 65 + DH] = b_qkv[src : src + DH]
            bv[0, j * 65 + DH] = 1.0
        bqk = np.concatenate(
            [b_qkv[h0 * DH : (h0 + GH) * DH], b_qkv[C + h0 * DH : C + (h0 + GH) * DH]]
        ).reshape(1, QK_COLS)
        wproj = np.stack(
            [w_proj[(h0 + 2 * p) * DH : (h0 + 2 * p + 2) * DH, :] for p in range(2)]
        )
        in_maps.append(
            {
                "xT": xTs[b],
                "wqk": wqk,
                "wv": _bf(wv),
                "bqk": _bf(bqk),
                "bv": _bf(bv),
                "cosT": cos2,
                "sinTp": sinp2,
                "perm": perm,
                "maskT": maskT,
                "id128": id128,
                "wproj": _bf(wproj),
            }
        )
    return in_maps


def run(x, w_qkv, b_qkv, w_proj, b_proj, trace=False, tmpdir=None):
    nc = _get_nc()
    in_maps = _prepare_in_maps(x, w_qkv, b_qkv, w_proj)
    res = run_bass_kernel_spmd(
        nc, in_maps, list(range(N_CORES)), trace=trace, tmpdir=tmpdir
    )
    b_proj = np.asarray(b_proj, dtype=np.float32)
    out = np.empty((B, T, C), dtype=np.float32)
    for b in range(B):
        acc = res.results[4 * b]["out"].astype(np.float32)
        for g in range(1, 4):
            acc = acc + res.results[4 * b + g]["out"]
        out[b] = acc + b_proj
    return out, res


def kernel(x, w_qkv, b_qkv, w_proj, b_proj):
    out, _ = run(x, w_qkv, b_qkv, w_proj, b_proj, trace=False)
    return out
